# revision 9
# baseline (speedup 1.0000x reference)
"""Trainium2 Bass kernel for nn_ClassicalHybridClassifier.

Pipeline: conv1(5x5,s2) -> maxpool(2,s1) -> conv2(3x3,s2) -> maxpool(2,s1)
          -> fc1 [120,55815] -> fc2 -> fc3 -> qnn tanh stack -> RBF vs 8192
          train states -> [1,2] output.

Sharding: each of the 8 cores computes a horizontal band of the conv pipeline
(bands over the 61 pool2 output rows: 8,8,8,8,8,7,7,7) and the matching
contraction slice of fc1 (tensor-parallel over fc1's 55815 input dim, weights
restructured host-side to match the on-chip feature layout). The [10,120] fc1
partials are AllGathered (cheaper than AllReduce) and summed on-chip with a
selection-matrix matmul that simultaneously transposes to [120,10]; the tiny
tail (fc2/fc3/qnn/RBF over all 8192 train states) is replicated on every core.

The whole conv pipeline runs in bf16 (fp32 PSUM accumulation): measured final
rel err ~7.6e-4, dominated by the bf16 fc1 weights which the fp32 variant also
rounds. Convs are banded-weight matmuls: contraction over (channel, input row)
with kernel-column taps accumulated in PSUM via column-shifted strided views.
Vertical max-pools cross the partition dim, handled by partition-shift matmuls.
Inputs stream over the two HWDGE queues (sync + scalar), chunked so conv1
starts before the full load; per-chunk PSUM tiles let chunks pipeline.
"""

import numpy as np
import ml_dtypes

import concourse.bass as bass
import concourse.mybir as mybir
import concourse.tile as tile
from concourse import bass_utils, bacc

F32 = mybir.dt.float32
BF16 = mybir.dt.bfloat16
AF = mybir.ActivationFunctionType
ALU = mybir.AluOpType
AX = mybir.AxisListType

N_CORES = 8
BANDS = [(0, 8), (8, 16), (16, 24), (24, 32), (32, 40), (40, 47), (47, 54), (54, 61)]

B = 10          # batch
XR = 43         # x rows per core (padded)
XC = 252        # x cols incl 1+1 zero pad
C1R = 20        # conv1 out rows per core (padded)
P1R = 19        # pool1 rows per core (padded)
C2R = 9         # conv2 out rows per core (padded)
NJ = 61         # pool2 / fc1 spatial columns
C1CH = 6

# image chunking (PSUM bank = 512 fp32)
C1_CHUNKS = [(0, 4), (4, 3), (7, 3)]     # conv1/pool1: 4*124=496, 3*124=372
C2_CHUNKS = [(0, 8), (8, 2)]             # conv2/pool2: 8*62=496, 2*62=124


def _build_nc():
    nc = bacc.Bacc("TRN2", target_bir_lowering=False, debug=False,
                   num_devices=N_CORES)

    d = {}
    def din(name, shape, dt):
        d[name] = nc.dram_tensor(name, list(shape), dt, kind="ExternalInput").ap()

    din("x2d", (87, B * XC), BF16)     # c0+c1 rows + ones row
    din("x3d", (43, B * XC), BF16)     # c2 rows
    din("wpk", (87, 1200), BF16)       # w1a | w1b
    din("pk2", (120, 768), BF16)       # s1m | s2a | s2b | w2
    din("wslab", (120, NJ, 120), BF16)
    din("pk1", (128, 592), F32)        # small fc/tail tensors + ssum
    din("ones_v", (1, B * 125), BF16)  # pool1 bias row (partition 114)

    out_d = nc.dram_tensor("out", [1, 2], F32, kind="ExternalOutput").ap()
    warm_d = nc.dram_tensor("warm", [1, 4], F32, kind="ExternalOutput").ap()

    with tile.TileContext(nc) as tc:
        with (
            tc.tile_pool(name="sb", bufs=1) as sb,
            tc.tile_pool(name="dr", bufs=1, space="DRAM") as dr,
        ):
            # ---- DMAs in (HWDGE queues; x chunks so conv1 starts early) ----
            x2e = sb.tile([87, B, XC], BF16)
            x3e = sb.tile([43, B, XC], BF16)
            wpk_t = sb.tile([87, 1200], BF16)
            pk2_t = sb.tile([120, 768], BF16)
            wslab_t = sb.tile([120, NJ, 120], BF16)
            pk1_t = sb.tile([128, 592], F32)

            # all big inputs on the sync HWDGE queue in priority order (the
            # scalar HWDGE queue measured ~3.4us per DMA instruction)
            nc.sync.dma_start(wpk_t[:], d["wpk"][:])
            for i0, ni in C1_CHUNKS:
                nc.sync.dma_start(x2e[:, i0:i0 + ni, :],
                                  d["x2d"][:, i0 * XC:(i0 + ni) * XC])
                nc.sync.dma_start(x3e[:, i0:i0 + ni, :],
                                  d["x3d"][:, i0 * XC:(i0 + ni) * XC])
            nc.sync.dma_start(wslab_t[:], d["wslab"][:])
            nc.gpsimd.dma_start(pk2_t[:], d["pk2"][:])
            nc.gpsimd.dma_start(pk1_t[:], d["pk1"][:])

            w1a_t = wpk_t[:, 0:600].rearrange("p (k m) -> p k m", m=120)
            w1b_t = wpk_t[0:43, 600:1200].rearrange("p (k m) -> p k m", m=120)

            s1m_t = pk2_t[0:120, 0:114]
            s2a_t = pk2_t[0:120, 114:234]
            s2b_t = pk2_t[0:15, 234:354]
            w2f = pk2_t[0:115, 354:759]          # [115, 3*135] flat

            small = {
                "fc1b": pk1_t[0:120, 0:1],
                "w2fcT": pk1_t[0:120, 1:85],
                "fc2b": pk1_t[0:84, 85:86],
                "w3fcT": pk1_t[0:84, 86:87],
                "b3vec": pk1_t[0:B, 87:88],
                "wq1T": pk1_t[0:B, 88:108],
                "wq2T": pk1_t[0:20, 108:113],
                "kclsb": pk1_t[0:1, 123:125],
                "ts_r": pk1_t[:, 128:448].rearrange("p (a b) -> p a b", b=5),
                "kcls_r": pk1_t[:, 448:576].rearrange("p (a b) -> p a b", b=64),
                "ssum": pk1_t[0:80, 576:586],
            }

            # constants / pads (vector engine, during input DMA)
            V = sb.tile([128, B, 125], BF16)     # pool1 out, (py,ich) + ones@114
            ones_row = sb.tile([1, 128], F32)
            ones_col = sb.tile([128, 1], F32)
            nc.vector.memset(ones_row[:], 1.0)
            nc.vector.memset(ones_col[:], 1.0)
            nc.vector.memset(V[0:114, :, 0:1], 0.0)
            nc.vector.memset(V[0:114, :, 124:125], 0.0)
            nc.gpsimd.dma_start(V[114:115, :, :],
                                d["ones_v"][:].rearrange("p (i c) -> p i c", c=125))

            Cs = sb.tile([120, B, 124], BF16)    # conv1 psum eviction
            Ch = sb.tile([120, B, 123], BF16)    # horizontal max
            C2s_a = sb.tile([120, B, 62], BF16)
            C2s_b = sb.tile([15, B, 62], BF16)
            C2h_a = sb.tile([120, B, 61], BF16)
            C2h_b = sb.tile([15, B, 61], BF16)
            V2 = sb.tile([120, B, NJ], BF16)

            with tc.tile_pool(name="ps_1", bufs=1, space="PSUM") as ps1:
                # PE warmup during input DMA (HAM un-throttle)
                wsc = sb.tile([128, 512], BF16)
                nc.vector.memset(wsc[:], 0.0)
                # long enough that the PE never idles a full HAM window
                # before conv1 data lands (else it re-throttles to 1.2 GHz)
                wps = ps1.tile([128, 512], F32)
                NWARM = 16
                for i in range(NWARM):
                    nc.tensor.matmul(wps[:], wsc[:, 0:128], wsc[:],
                                     start=(i == 0), stop=(i == NWARM - 1))
                wout = sb.tile([1, 4], F32)
                nc.vector.tensor_copy(wout[:], wps[0:1, 0:4])
                nc.sync.dma_start(warm_d[:], wout[:])

                # ---- conv1 + pool1 (per-chunk PSUM tiles pipeline) ----
                cps = [ps1.tile([120, ni * 124], F32, name=f"cp{i}")
                       for i, (_, ni) in enumerate(C1_CHUNKS)]
                shs = [ps1.tile([114, ni * 123], F32, name=f"sh{i}")
                       for i, (_, ni) in enumerate(C1_CHUNKS)]

                for ci, (i0, ni) in enumerate(C1_CHUNKS):
                    k = 0
                    for kx in range(5):
                        for grp in range(2):
                            xt, wt, kdim = ((x2e, w1a_t, 87) if grp == 0
                                            else (x3e, w1b_t, 43))
                            rhs = xt[0:kdim, i0:i0 + ni, kx:kx + 248:2]
                            nc.tensor.matmul(
                                cps[ci][:], wt[0:kdim, kx, :], rhs,
                                start=(k == 0), stop=(k == 9))
                            k += 1
                    cv = cps[ci][:].rearrange("p (i x) -> p i x", x=124)
                    # evict on ACT, then horizontal pool max (one PSUM operand)
                    nc.scalar.copy(Cs[:, i0:i0 + ni, :], cv)
                    nc.vector.tensor_max(Ch[:, i0:i0 + ni, :],
                                         Cs[:, i0:i0 + ni, 0:123],
                                         cv[:, :, 1:124])
                    # vertical pool via partition-shift matmul; V = max(Ch,0,Sh)
                    nc.tensor.matmul(shs[ci][:], s1m_t[:],
                                     Ch[:, i0:i0 + ni, :],
                                     start=True, stop=True)
                    sv = shs[ci][:].rearrange("p (i x) -> p i x", x=123)
                    nc.vector.scalar_tensor_tensor(
                        V[0:114, i0:i0 + ni, 1:124],
                        Ch[0:114, i0:i0 + ni, :], 0.0, sv[0:114, :, :],
                        op0=ALU.max, op1=ALU.max)

            # ---- conv2 + pool2 ----
            with tc.tile_pool(name="ps_2", bufs=1, space="PSUM") as ps2:
                c2as = [ps2.tile([120, ni * 62], F32, name=f"c2a{i}")
                        for i, (_, ni) in enumerate(C2_CHUNKS)]
                c2bs = [ps2.tile([15, ni * 62], F32, name=f"c2b{i}")
                        for i, (_, ni) in enumerate(C2_CHUNKS)]
                sh2s = [ps2.tile([120, ni * 61], F32, name=f"sh2{i}")
                        for i, (_, ni) in enumerate(C2_CHUNKS)]

                for ci, (i0, ni) in enumerate(C2_CHUNKS):
                    for grp, (cp, m0, m1) in enumerate(
                            ((c2as[ci], 0, 120), (c2bs[ci], 120, 135))):
                        for kxp in range(3):
                            rhs = V[0:115, i0:i0 + ni, kxp:kxp + 123:2]
                            nc.tensor.matmul(
                                cp[:],
                                w2f[:, kxp * 135 + m0: kxp * 135 + m1], rhs,
                                start=(kxp == 0), stop=(kxp == 2))
                    for cp, cs, ch in ((c2as[ci], C2s_a, C2h_a),
                                       (c2bs[ci], C2s_b, C2h_b)):
                        cv = cp[:].rearrange("p (i x) -> p i x", x=62)
                        nc.scalar.copy(cs[:, i0:i0 + ni, :], cv)
                        # relu + horizontal pool (one PSUM operand)
                        nc.vector.scalar_tensor_tensor(
                            ch[:, i0:i0 + ni, :],
                            cs[:, i0:i0 + ni, 0:61], 0.0, cv[:, :, 1:62],
                            op0=ALU.max, op1=ALU.max)
                    nc.tensor.matmul(sh2s[ci][:], s2a_t[:],
                                     C2h_a[:, i0:i0 + ni, :],
                                     start=True, stop=False)
                    nc.tensor.matmul(sh2s[ci][:], s2b_t[:],
                                     C2h_b[:, i0:i0 + ni, :],
                                     start=False, stop=True)
                    sv = sh2s[ci][:].rearrange("p (i x) -> p i x", x=61)
                    nc.vector.tensor_max(V2[:, i0:i0 + ni, :],
                                         C2h_a[:, i0:i0 + ni, :], sv)

            # ---- fc1 (single-pass bf16, tensor-parallel contraction) ----
            fc1s = sb.tile([B, 120], F32)
            with tc.tile_pool(name="ps_3", bufs=1, space="PSUM") as ps3:
                fps = ps3.tile([B, 120], F32)
                for j in range(NJ):
                    nc.tensor.matmul(fps[:], V2[:, :, j], wslab_t[:, j, :],
                                     start=(j == 0), stop=(j == NJ - 1))
                nc.vector.tensor_copy(fc1s[:], fps[:])

            # ---- AllGather fc1 partials (cheaper than AllReduce) ----
            arin = dr.tile([B, 120], F32)
            arout = dr.tile([N_CORES * B, 120], F32, addr_space="Shared")
            nc.sync.dma_start(arin[:], fc1s[:])
            nc.gpsimd.collective_compute(
                "AllGather", ALU.bypass,
                replica_groups=[list(range(N_CORES))],
                ins=[arin.opt()], outs=[arout.opt()])
            agsb = sb.tile([N_CORES * B, 120], F32)
            nc.sync.dma_start(agsb[:], arout[:])

            # ---- tail (replicated) ----
            h1 = sb.tile([120, B], F32)
            h2 = sb.tile([84, B], F32)
            h10 = sb.tile([B, 1], F32)
            s1t = sb.tile([20, 1], F32)
            fs_row = sb.tile([1, 5], F32)
            diff = sb.tile([128, 64, 5], F32)
            sq = sb.tile([128, 64, 5], F32)
            d2 = sb.tile([128, 64], F32)
            kxv = sb.tile([128, 64], F32)
            pr = sb.tile([128, 2, 64], F32)
            krw = sb.tile([128, 2], F32)
            out_sb = sb.tile([1, 2], F32)

            with tc.tile_pool(name="ps_4", bufs=1, space="PSUM") as ps4:
                # sum the 8 gathered partials; the selection matrix also
                # transposes [80,120] -> [120,10]
                tp = ps4.tile([120, B], F32)
                nc.tensor.matmul(tp[:], agsb[:], small["ssum"][:],
                                 start=True, stop=True)
                nc.scalar.activation(h1[:], tp[:], AF.Relu,
                                     bias=small["fc1b"][:])

                p2 = ps4.tile([84, B], F32)
                nc.tensor.matmul(p2[:], small["w2fcT"][:], h1[:],
                                 start=True, stop=True)
                nc.scalar.activation(h2[:], p2[:], AF.Relu,
                                     bias=small["fc2b"][:])

                p3 = ps4.tile([B, 1], F32)
                nc.tensor.matmul(p3[:], h2[:], small["w3fcT"][:],
                                 start=True, stop=True)
                nc.scalar.activation(h10[:], p3[:], AF.Identity,
                                     bias=small["b3vec"][:])

                p4 = ps4.tile([20, 1], F32)
                nc.tensor.matmul(p4[:], small["wq1T"][:], h10[:],
                                 start=True, stop=True)
                nc.scalar.activation(s1t[:], p4[:], AF.Tanh)

                p5 = ps4.tile([1, 5], F32)
                nc.tensor.matmul(p5[:], s1t[:], small["wq2T"][:],
                                 start=True, stop=True)
                nc.scalar.activation(fs_row[:], p5[:], AF.Tanh)

                # broadcast fs to 128 partitions via K=1 matmul
                pb = ps4.tile([128, 5], F32)
                nc.tensor.matmul(pb[:], ones_row[:], fs_row[:],
                                 start=True, stop=True)
                nc.vector.tensor_sub(
                    diff[:], small["ts_r"][:],
                    pb[:].unsqueeze(1).broadcast_to([128, 64, 5]))
                nc.vector.tensor_mul(sq[:], diff[:], diff[:])
                nc.vector.reduce_sum(d2[:], sq[:], axis=AX.X)
                nc.scalar.activation(kxv[:], d2[:], AF.Exp, scale=-1.0)
                nc.vector.tensor_mul(
                    pr[:], small["kcls_r"][:],
                    kxv[:].unsqueeze(1).broadcast_to([128, 2, 64]))
                nc.vector.reduce_sum(krw[:], pr[:], axis=AX.X)

                p6 = ps4.tile([1, 2], F32)
                nc.tensor.matmul(p6[:], ones_col[:], krw[:],
                                 start=True, stop=True)
                nc.vector.tensor_add(out_sb[:], p6[:], small["kclsb"][:])

            nc.sync.dma_start(out_d[:], out_sb[:])

    nc.compile()
    return nc


def _prep_inputs(inputs):
    f32 = np.float32
    bf = ml_dtypes.bfloat16
    x = np.asarray(inputs["x"], f32)
    conv1_w = np.asarray(inputs["conv1_w"], f32)
    conv1_b = np.asarray(inputs["conv1_b"], f32)
    conv2_w = np.asarray(inputs["conv2_w"], f32)
    conv2_b = np.asarray(inputs["conv2_b"], f32)
    fc1_w = np.asarray(inputs["fc1_w"], f32)
    fc1_b = np.asarray(inputs["fc1_b"], f32)
    fc2_w = np.asarray(inputs["fc2_w"], f32)
    fc2_b = np.asarray(inputs["fc2_b"], f32)
    fc3_w = np.asarray(inputs["fc3_w"], f32)
    fc3_b = np.asarray(inputs["fc3_b"], f32)
    qnn_w1 = np.asarray(inputs["qnn_w1"], f32)
    qnn_w2 = np.asarray(inputs["qnn_w2"], f32)
    ts = np.asarray(inputs["train_states"], f32)
    kcls_w = np.asarray(inputs["kcls_w"], f32)
    kcls_b = np.asarray(inputs["kcls_b"], f32)

    pk1 = np.zeros((128, 592), f32)
    pk1[0:120, 0:1] = fc1_b.reshape(120, 1)
    pk1[0:120, 1:85] = fc2_w.T
    pk1[0:84, 85:86] = fc2_b.reshape(84, 1)
    pk1[0:84, 86:87] = fc3_w.T
    pk1[0:B, 87:88] = fc3_b[0]
    pk1[0:B, 88:108] = qnn_w1.T
    pk1[0:20, 108:113] = qnn_w2.T
    pk1[0:1, 123:125] = kcls_b.reshape(1, 2)
    pk1[:, 128:448] = ts.reshape(128, 320)
    pk1[:, 448:576] = kcls_w.reshape(2, 128, 64).transpose(1, 0, 2).reshape(128, 128)
    ssum = np.zeros((80, 10), f32)
    for r in range(N_CORES):
        for b in range(B):
            ssum[r * B + b, b] = 1.0
    pk1[0:80, 576:586] = ssum
    shared = {"pk1": pk1,
              "ones_v": np.ones((1, B * 125), ml_dtypes.bfloat16)}

    fc1_w4 = fc1_w.reshape(120, 15, 61, 61)

    in_maps = []
    for a, b in BANDS:
        nb = b - a
        Y0 = 2 * a - 1          # conv1 row of y_loc 0 (also pool1 row of py_loc 0)
        X0 = 4 * a - 3          # x row of r_loc 0

        # x slabs: x2 = [c0 rows | c1 rows | ones], x3 = [c2 rows]
        xs = np.zeros((3, XR, B, XC), f32)
        r_lo = max(0, X0)
        r_hi = min(250, X0 + XR)
        xs[:, r_lo - X0: r_hi - X0, :, 1:251] = (
            x[:, :, r_lo:r_hi, :].transpose(1, 2, 0, 3))
        x2 = np.concatenate(
            [xs[0], xs[1], np.ones((1, B, XC), f32)], axis=0)
        x3 = xs[2]

        # conv1 banded weights: K=(c, r_loc)+bias, M=(y_loc, och), per kx
        w1 = np.zeros((3, 43, 5, 120), f32)     # [c, r_loc, kx, m=(y_loc,och)]
        for y_loc in range(C1R):
            y = Y0 + y_loc
            if not (0 <= y <= 123):
                continue
            for ky in range(5):
                r_loc = 2 * y_loc + ky
                if r_loc >= XR:
                    continue
                for c in range(3):
                    w1[c, r_loc, :, y_loc * 6: y_loc * 6 + 6] = \
                        conv1_w[:, c, ky, :].T
        w1a = np.zeros((87, 5, 120), f32)
        w1a[0:43] = w1[0]
        w1a[43:86] = w1[1]
        w1a[86, 0, :] = np.tile(conv1_b, C1R)   # bias row, kx=0 only
        w1b = np.ascontiguousarray(w1[2])

        # conv2 banded weights: K=(py_loc, ich)+bias@114, M=(i2_loc, och2)
        w2 = np.zeros((115, 3, 135), f32)
        for i2_loc in range(C2R):
            i2 = a + i2_loc
            if i2 > 61:
                continue
            for kyp in range(3):
                py_loc = 2 * i2_loc + kyp
                py = Y0 + py_loc
                if py_loc >= P1R or not (0 <= py <= 122):
                    continue
                for ich in range(6):
                    q = py_loc * 6 + ich
                    m0 = i2_loc * 15
                    w2[q, :, m0:m0 + 15] = conv2_w[:, ich, kyp, :].T
        w2[114, 0, :] = np.tile(conv2_b, 9)     # bias row, kxp=0 only

        # partition-shift matrices
        s1m = np.zeros((120, 114), f32)
        for m in range(114):
            s1m[m + 6, m] = 1.0
        s2a = np.zeros((120, 120), f32)
        s2b = np.zeros((15, 120), f32)
        for m in range(105):
            s2a[m + 15, m] = 1.0
        for m in range(105, 120):
            s2b[m - 105, m] = 1.0

        # fc1 weight slab: [p=(i2_loc,och2), j, och1]
        wsl = np.zeros((8, 15, NJ, 120), f32)
        nrow = min(nb, 8)
        wsl[0:nrow] = fc1_w4[:, :, a:a + nrow, :].transpose(2, 1, 3, 0)
        wslab = wsl.reshape(120, NJ, 120).astype(bf)

        wpk = np.zeros((87, 1200), f32)
        wpk[:, 0:600] = w1a.reshape(87, 600)
        wpk[0:43, 600:1200] = w1b.reshape(43, 600)

        pk2 = np.zeros((120, 768), f32)
        pk2[0:120, 0:114] = s1m
        pk2[0:120, 114:234] = s2a
        pk2[0:15, 234:354] = s2b
        pk2[0:115, 354:759] = w2.reshape(115, 405)

        m = dict(shared)
        m.update({"x2d": np.ascontiguousarray(x2.reshape(87, B * XC)).astype(bf),
                  "x3d": np.ascontiguousarray(x3.reshape(43, B * XC)).astype(bf),
                  "wpk": wpk.astype(bf),
                  "pk2": pk2.astype(bf),
                  "wslab": np.ascontiguousarray(wslab)})
        in_maps.append(m)
    return in_maps


_NC_CACHE = None


def kernel(**inputs) -> np.ndarray:
    global _NC_CACHE
    if _NC_CACHE is None:
        _NC_CACHE = _build_nc()
    nc = _NC_CACHE
    in_maps = _prep_inputs(inputs)
    res = bass_utils.run_bass_kernel_spmd(
        nc, in_maps, core_ids=list(range(N_CORES)))
    return res.results[0]["out"]


# revision 25
# speedup vs baseline: 1.0653x; 1.0653x over previous
"""Trainium2 Bass kernel for nn_ClassicalHybridClassifier.

Pipeline: conv1(5x5,s2) -> maxpool(2,s1) -> conv2(3x3,s2) -> maxpool(2,s1)
          -> fc1 [120,55815] -> fc2 -> fc3 -> qnn tanh stack -> RBF vs 8192
          train states -> [1,2] output.

Sharding: each of the 8 cores computes a horizontal band of the conv pipeline
(bands over the 61 pool2 output rows: 8,8,8,8,8,7,7,7) and the matching
contraction slice of fc1 (tensor-parallel over fc1's 55815 input dim, weights
restructured host-side to match the on-chip feature layout). The [10,120] fc1
partials are AllGathered on the free dim (cheaper than AllReduce) and summed
on-chip; the tiny tail (fc2/fc3/qnn/RBF over 8192 train states) is replicated.

All-bf16 pipeline (fp32 PSUM accumulation): final rel err ~3e-3, dominated by
the bf16 fc1 weights which the fp32 variant also rounds.  DMA cost on this
part is per-descriptor (~40ns per partition-row), so inputs are packed into
few wide tensors: x + conv1 weights ride in one [128]+[1]-partition pair
(conv biases are folded into the PSUM-eviction activations, which frees the
ones/bias rows and lets 3x43 input rows fit 128+1 partitions), and
pool-shift/conv2/fc1 weights share one [120]-row tensor.  Each big DMA is
split across the two HWDGE queues.  A long PE warmup keeps the HAM clock
gate open (2.4 GHz) until conv1 data lands.
"""

import numpy as np
import ml_dtypes

import concourse.bass as bass
import concourse.mybir as mybir
import concourse.tile as tile
from concourse import bass_utils, bacc

F32 = mybir.dt.float32
BF16 = mybir.dt.bfloat16
AF = mybir.ActivationFunctionType
ALU = mybir.AluOpType
AX = mybir.AxisListType

N_CORES = 8
BANDS = [(0, 8), (8, 16), (16, 24), (24, 32), (32, 40), (40, 47), (47, 54), (54, 61)]

B = 10          # batch
XR = 43         # x rows per core per channel (padded)
XC = 252        # x cols incl 1+1 zero pad
C1R = 20        # conv1 out rows per core (padded)
P1R = 19        # pool1 rows per core (padded)
C2R = 9         # conv2 out rows per core (padded)
NJ = 61         # pool2 / fc1 spatial columns

XW = 3120       # xall cols: B*XC x data + 5*120 weights
W2W = 8088      # big2 cols: 768 (shift mats + w2) + 61*120 (fc1 slab)

# image chunking (PSUM bank = 512 fp32)
C1_CHUNKS = [(0, 4), (4, 3), (7, 3)]     # conv1/pool1: 4*124=496, 3*124=372
C2_CHUNKS = [(0, 8), (8, 2)]             # conv2/pool2: 8*62=496, 2*62=124


def _build_nc():
    nc = bacc.Bacc("TRN2", target_bir_lowering=False, debug=False,
                   num_devices=N_CORES)

    d = {}
    def din(name, shape, dt):
        d[name] = nc.dram_tensor(name, list(shape), dt, kind="ExternalInput").ap()

    din("xall", (128, XW), BF16)       # x rows (c0|c1|c2[0:42]) + conv1 w
    din("xlast", (1, XW), BF16)        # c2 row 42 + its conv1 w
    din("big2", (120, W2W), BF16)      # s1m | s2a | s2b | w2 | fc1 slab
    din("pk1", (128, 592), F32)        # small fc/tail tensors + biases

    out_d = nc.dram_tensor("out", [1, 2], F32, kind="ExternalOutput").ap()
    warm_d = nc.dram_tensor("warm", [1, 4], F32, kind="ExternalOutput").ap()

    with tile.TileContext(nc) as tc:
        with (
            tc.tile_pool(name="sb", bufs=1) as sb,
            tc.tile_pool(name="dr", bufs=1, space="DRAM") as dr,
        ):
            xall_t = sb.tile([128, XW], BF16)
            xlast_t = sb.tile([1, XW], BF16)
            big2_t = sb.tile([120, W2W], BF16)
            pk1_t = sb.tile([128, 592], F32)

            # split each big DMA across the two HWDGE queues (cost is
            # ~40ns/partition-row per queue, bytes are nearly free)
            # scalar HWDGE drains ~4x slower than sync with high variance, so
            # it gets only a small x slice; gpsimd SWDGE takes big2's tail
            nc.sync.dma_start(xall_t[0:84, :], d["xall"][0:84, :])
            nc.scalar.dma_start(xall_t[84:128, :], d["xall"][84:128, :])
            nc.sync.dma_start(xlast_t[:], d["xlast"][:])
            nc.sync.dma_start(big2_t[0:80, :], d["big2"][0:80, :])
            nc.gpsimd.dma_start(big2_t[80:120, :], d["big2"][80:120, :])
            nc.gpsimd.dma_start(pk1_t[:], d["pk1"][:])

            x_a = xall_t[:, 0:B * XC].rearrange("p (i c) -> p i c", c=XC)
            x_b = xlast_t[:, 0:B * XC].rearrange("p (i c) -> p i c", c=XC)
            wA = xall_t[:, B * XC:XW].rearrange("p (k m) -> p k m", m=120)
            wB = xlast_t[:, B * XC:XW].rearrange("p (k m) -> p k m", m=120)

            s1m_t = big2_t[0:120, 0:114]
            s2a_t = big2_t[0:120, 114:234]
            s2b_t = big2_t[0:15, 234:354]
            w2f = big2_t[0:114, 354:759]         # [114, 3*135] flat
            wslab_t = big2_t[:, 768:W2W].rearrange("p (j m) -> p j m", m=120)

            small = {
                "fc1b": pk1_t[0:120, 0:1],
                "w2fcT": pk1_t[0:120, 1:85],
                "fc2b": pk1_t[0:84, 85:86],
                "w3fcT": pk1_t[0:84, 86:87],
                "b3vec": pk1_t[0:B, 87:88],
                "wq1T": pk1_t[0:B, 88:108],
                "wq2T": pk1_t[0:20, 108:113],
                "idt10": pk1_t[0:B, 113:123],
                "kclsb": pk1_t[0:1, 123:125],
                "bias1": pk1_t[0:120, 125:126],
                "b2a": pk1_t[0:120, 126:127],
                "b2b": pk1_t[0:15, 127:128],
                "ts_r": pk1_t[:, 128:448].rearrange("p (a b) -> p a b", b=5),
                "kcls_r": pk1_t[:, 448:576].rearrange("p (a b) -> p a b", b=64),
                "ssum": pk1_t[0:80, 576:586],
            }

            # constants / pads (vector engine, during input DMA)
            V = sb.tile([128, B, 125], BF16)     # pool1 out, (py,ich)
            ones_row = sb.tile([1, 128], F32)
            ones_col = sb.tile([128, 1], F32)
            nc.vector.memset(ones_row[:], 1.0)
            nc.vector.memset(ones_col[:], 1.0)
            nc.vector.memset(V[0:114, :, 0:1], 0.0)
            nc.vector.memset(V[0:114, :, 124:125], 0.0)

            Cs = sb.tile([120, B, 124], BF16)    # conv1 evict (+bias)
            Ch = sb.tile([120, B, 123], BF16)    # horizontal max
            C2s_a = sb.tile([120, B, 62], BF16)
            C2s_b = sb.tile([15, B, 62], BF16)
            C2h_a = sb.tile([120, B, 61], BF16)
            C2h_b = sb.tile([15, B, 61], BF16)
            V2 = sb.tile([120, B, NJ], BF16)

            with tc.tile_pool(name="ps_1", bufs=1, space="PSUM") as ps1:
                # PE warmup: long enough that the PE never idles a full HAM
                # window before conv1 data lands (else it drops to 1.2 GHz)
                wsc = sb.tile([128, 512], BF16)
                nc.vector.memset(wsc[:], 0.0)
                wps = ps1.tile([128, 512], F32)
                NWARM = 27
                for i in range(NWARM):
                    nc.tensor.matmul(wps[:], wsc[:, 0:128], wsc[:],
                                     start=(i == 0), stop=(i == NWARM - 1))
                wout = sb.tile([1, 4], F32)
                nc.vector.tensor_copy(wout[:], wps[0:1, 0:4])
                nc.sync.dma_start(warm_d[:], wout[:])

                # ---- conv1 + pool1 (per-chunk PSUM tiles pipeline) ----
                cps = [ps1.tile([120, ni * 124], F32, name=f"cp{i}")
                       for i, (_, ni) in enumerate(C1_CHUNKS)]
                shs = [ps1.tile([114, ni * 123], F32, name=f"sh{i}")
                       for i, (_, ni) in enumerate(C1_CHUNKS)]

                def sh1_pool(ci):
                    # vertical pool via partition-shift matmul; V = max(Ch,0,Sh)
                    i0, ni = C1_CHUNKS[ci]
                    nc.tensor.matmul(shs[ci][:], s1m_t[:],
                                     Ch[:, i0:i0 + ni, :],
                                     start=True, stop=True)
                    sv = shs[ci][:].rearrange("p (i x) -> p i x", x=123)
                    nc.vector.scalar_tensor_tensor(
                        V[0:114, i0:i0 + ni, 1:124],
                        Ch[0:114, i0:i0 + ni, :], 0.0, sv[0:114, :, :],
                        op0=ALU.max, op1=ALU.max)

                for ci, (i0, ni) in enumerate(C1_CHUNKS):
                    for kx in range(5):
                        nc.tensor.matmul(
                            cps[ci][:], wA[:, kx, :],
                            x_a[:, i0:i0 + ni, kx:kx + 248:2],
                            start=(kx == 0), stop=False)
                        nc.tensor.matmul(
                            cps[ci][:], wB[:, kx, :],
                            x_b[:, i0:i0 + ni, kx:kx + 248:2],
                            start=False, stop=(kx == 4))
                    cv = cps[ci][:].rearrange("p (i x) -> p i x", x=124)
                    # evict on ACT with conv1 bias folded in
                    nc.scalar.activation(Cs[:, i0:i0 + ni, :], cv, AF.Identity,
                                         bias=small["bias1"][:])
                    nc.vector.tensor_max(Ch[:, i0:i0 + ni, :],
                                         Cs[:, i0:i0 + ni, 0:123],
                                         Cs[:, i0:i0 + ni, 1:124])
                    # shift-matmul lags one chunk so the PE never waits on
                    # the eviction chain mid-stream
                    if ci >= 1:
                        sh1_pool(ci - 1)
                sh1_pool(len(C1_CHUNKS) - 1)

            # ---- conv2 + pool2 ----
            with tc.tile_pool(name="ps_2", bufs=1, space="PSUM") as ps2:
                c2as = [ps2.tile([120, ni * 62], F32, name=f"c2a{i}")
                        for i, (_, ni) in enumerate(C2_CHUNKS)]
                c2bs = [ps2.tile([15, ni * 62], F32, name=f"c2b{i}")
                        for i, (_, ni) in enumerate(C2_CHUNKS)]
                sh2s = [ps2.tile([120, ni * 61], F32, name=f"sh2{i}")
                        for i, (_, ni) in enumerate(C2_CHUNKS)]

                def sh2_pool(ci):
                    i0, ni = C2_CHUNKS[ci]
                    nc.tensor.matmul(sh2s[ci][:], s2a_t[:],
                                     C2h_a[:, i0:i0 + ni, :],
                                     start=True, stop=False)
                    nc.tensor.matmul(sh2s[ci][:], s2b_t[:],
                                     C2h_b[:, i0:i0 + ni, :],
                                     start=False, stop=True)
                    sv = sh2s[ci][:].rearrange("p (i x) -> p i x", x=61)
                    nc.vector.tensor_max(V2[:, i0:i0 + ni, :],
                                         C2h_a[:, i0:i0 + ni, :], sv)

                for ci, (i0, ni) in enumerate(C2_CHUNKS):
                    for cp, m0, m1 in ((c2as[ci], 0, 120),
                                       (c2bs[ci], 120, 135)):
                        for kxp in range(3):
                            rhs = V[0:114, i0:i0 + ni, kxp:kxp + 123:2]
                            nc.tensor.matmul(
                                cp[:],
                                w2f[:, kxp * 135 + m0: kxp * 135 + m1], rhs,
                                start=(kxp == 0), stop=(kxp == 2))
                    for cp, cs, ch, bias in (
                            (c2as[ci], C2s_a, C2h_a, small["b2a"]),
                            (c2bs[ci], C2s_b, C2h_b, small["b2b"])):
                        cv = cp[:].rearrange("p (i x) -> p i x", x=62)
                        nc.scalar.activation(cs[:, i0:i0 + ni, :], cv,
                                             AF.Identity, bias=bias[:])
                        # relu + horizontal pool
                        nc.vector.scalar_tensor_tensor(
                            ch[:, i0:i0 + ni, :],
                            cs[:, i0:i0 + ni, 0:61], 0.0,
                            cs[:, i0:i0 + ni, 1:62],
                            op0=ALU.max, op1=ALU.max)
                    if ci >= 1:
                        sh2_pool(ci - 1)
                sh2_pool(len(C2_CHUNKS) - 1)

            # ---- fc1 (single-pass bf16, tensor-parallel contraction) ----
            fc1s = sb.tile([B, 120], F32)
            with tc.tile_pool(name="ps_3", bufs=1, space="PSUM") as ps3:
                fps = ps3.tile([B, 120], F32)
                for j in range(NJ):
                    nc.tensor.matmul(fps[:], V2[:, :, j], wslab_t[:, j, :],
                                     start=(j == 0), stop=(j == NJ - 1))
                nc.vector.tensor_copy(fc1s[:], fps[:])

            # ---- AllGather fc1 partials (partition-block layout) ----
            arin = dr.tile([B, 120], F32)
            arout = dr.tile([N_CORES * B, 120], F32, addr_space="Shared")
            nc.sync.dma_start(arin[:], fc1s[:])
            nc.gpsimd.collective_compute(
                "AllGather", ALU.bypass,
                replica_groups=[list(range(N_CORES))],
                ins=[arin.opt()], outs=[arout.opt()])
            agsb = sb.tile([N_CORES * B, 120], F32)
            nc.sync.dma_start(agsb[0:40, :], arout[0:40, :])
            nc.scalar.dma_start(agsb[40:80, :], arout[40:80, :])

            # ---- tail (replicated) ----
            h1 = sb.tile([120, B], F32)
            h2 = sb.tile([84, B], F32)
            h10 = sb.tile([B, 1], F32)
            s1t = sb.tile([20, 1], F32)
            fs_row = sb.tile([1, 5], F32)
            diff = sb.tile([128, 64, 5], F32)
            sq = sb.tile([128, 64, 5], F32)
            d2 = sb.tile([128, 64], F32)
            kxv = sb.tile([128, 64], F32)
            pr = sb.tile([128, 2, 64], F32)
            krw = sb.tile([128, 2], F32)
            out_sb = sb.tile([1, 2], F32)

            with tc.tile_pool(name="ps_4", bufs=1, space="PSUM") as ps4:
                # the selection matrix sums the 8 rank blocks and
                # simultaneously transposes [80,120] -> [120,10]
                tp = ps4.tile([120, B], F32)
                nc.tensor.matmul(tp[:], agsb[:], small["ssum"][:],
                                 start=True, stop=True)
                nc.scalar.activation(h1[:], tp[:], AF.Relu,
                                     bias=small["fc1b"][:])

                p2 = ps4.tile([84, B], F32)
                nc.tensor.matmul(p2[:], small["w2fcT"][:], h1[:],
                                 start=True, stop=True)
                nc.scalar.activation(h2[:], p2[:], AF.Relu,
                                     bias=small["fc2b"][:])

                p3 = ps4.tile([B, 1], F32)
                nc.tensor.matmul(p3[:], h2[:], small["w3fcT"][:],
                                 start=True, stop=True)
                nc.scalar.activation(h10[:], p3[:], AF.Identity,
                                     bias=small["b3vec"][:])

                p4 = ps4.tile([20, 1], F32)
                nc.tensor.matmul(p4[:], small["wq1T"][:], h10[:],
                                 start=True, stop=True)
                nc.scalar.activation(s1t[:], p4[:], AF.Tanh)

                p5 = ps4.tile([1, 5], F32)
                nc.tensor.matmul(p5[:], s1t[:], small["wq2T"][:],
                                 start=True, stop=True)
                nc.scalar.activation(fs_row[:], p5[:], AF.Tanh)

                # broadcast fs to 128 partitions via K=1 matmul
                pb = ps4.tile([128, 5], F32)
                nc.tensor.matmul(pb[:], ones_row[:], fs_row[:],
                                 start=True, stop=True)
                nc.vector.tensor_sub(
                    diff[:], small["ts_r"][:],
                    pb[:].unsqueeze(1).broadcast_to([128, 64, 5]))
                nc.vector.tensor_mul(sq[:], diff[:], diff[:])
                nc.vector.reduce_sum(d2[:], sq[:], axis=AX.X)
                nc.scalar.activation(kxv[:], d2[:], AF.Exp, scale=-1.0)
                nc.vector.tensor_mul(
                    pr[:], small["kcls_r"][:],
                    kxv[:].unsqueeze(1).broadcast_to([128, 2, 64]))
                nc.vector.reduce_sum(krw[:], pr[:], axis=AX.X)

                p6 = ps4.tile([1, 2], F32)
                nc.tensor.matmul(p6[:], ones_col[:], krw[:],
                                 start=True, stop=True)
                nc.vector.tensor_add(out_sb[:], p6[:], small["kclsb"][:])

            nc.sync.dma_start(out_d[:], out_sb[:])

    nc.compile()
    return nc


def _prep_inputs(inputs):
    f32 = np.float32
    bf = ml_dtypes.bfloat16
    x = np.asarray(inputs["x"], f32)
    conv1_w = np.asarray(inputs["conv1_w"], f32)
    conv1_b = np.asarray(inputs["conv1_b"], f32)
    conv2_w = np.asarray(inputs["conv2_w"], f32)
    conv2_b = np.asarray(inputs["conv2_b"], f32)
    fc1_w = np.asarray(inputs["fc1_w"], f32)
    fc1_b = np.asarray(inputs["fc1_b"], f32)
    fc2_w = np.asarray(inputs["fc2_w"], f32)
    fc2_b = np.asarray(inputs["fc2_b"], f32)
    fc3_w = np.asarray(inputs["fc3_w"], f32)
    fc3_b = np.asarray(inputs["fc3_b"], f32)
    qnn_w1 = np.asarray(inputs["qnn_w1"], f32)
    qnn_w2 = np.asarray(inputs["qnn_w2"], f32)
    ts = np.asarray(inputs["train_states"], f32)
    kcls_w = np.asarray(inputs["kcls_w"], f32)
    kcls_b = np.asarray(inputs["kcls_b"], f32)

    pk1 = np.zeros((128, 592), f32)
    pk1[0:120, 0:1] = fc1_b.reshape(120, 1)
    pk1[0:120, 1:85] = fc2_w.T
    pk1[0:84, 85:86] = fc2_b.reshape(84, 1)
    pk1[0:84, 86:87] = fc3_w.T
    pk1[0:B, 87:88] = fc3_b[0]
    pk1[0:B, 88:108] = qnn_w1.T
    pk1[0:20, 108:113] = qnn_w2.T
    pk1[0:B, 113:123] = np.eye(B, dtype=f32)
    pk1[0:1, 123:125] = kcls_b.reshape(1, 2)
    pk1[0:120, 125:126] = np.tile(conv1_b, C1R).reshape(120, 1)
    pk1[0:120, 126:127] = np.tile(conv2_b, 8).reshape(120, 1)
    pk1[0:15, 127:128] = conv2_b.reshape(15, 1)
    pk1[:, 128:448] = ts.reshape(128, 320)
    pk1[:, 448:576] = kcls_w.reshape(2, 128, 64).transpose(1, 0, 2).reshape(128, 128)
    for r in range(N_CORES):
        for bb in range(B):
            pk1[r * B + bb, 576 + bb] = 1.0
    shared = {"pk1": pk1}

    fc1_w4 = fc1_w.reshape(120, 15, 61, 61)

    in_maps = []
    for a, b in BANDS:
        nb = b - a
        Y0 = 2 * a - 1          # conv1 row of y_loc 0 (also pool1 row of py_loc 0)
        X0 = 4 * a - 3          # x row of r_loc 0

        # x rows per channel, zero-padded at image borders
        xs = np.zeros((3, XR, B, XC), f32)
        r_lo = max(0, X0)
        r_hi = min(250, X0 + XR)
        xs[:, r_lo - X0: r_hi - X0, :, 1:251] = (
            x[:, :, r_lo:r_hi, :].transpose(1, 2, 0, 3))

        # conv1 banded weights: K=(c, r_loc), M=(y_loc, och), per kx; bias
        # folded into the PSUM eviction activation
        w1 = np.zeros((3, XR, 5, 120), f32)     # [c, r_loc, kx, m]
        for y_loc in range(C1R):
            y = Y0 + y_loc
            if not (0 <= y <= 123):
                continue
            for ky in range(5):
                r_loc = 2 * y_loc + ky
                if r_loc >= XR:
                    continue
                for c in range(3):
                    w1[c, r_loc, :, y_loc * 6: y_loc * 6 + 6] = \
                        conv1_w[:, c, ky, :].T

        # pack x + weights: partitions = c0[0:43] | c1[0:43] | c2[0:42],
        # leftover c2 row 42 in its own 1-partition tensor
        xall = np.zeros((128, XW), f32)
        xlast = np.zeros((1, XW), f32)
        xall[0:43, 0:B * XC] = xs[0].reshape(XR, B * XC)
        xall[43:86, 0:B * XC] = xs[1].reshape(XR, B * XC)
        xall[86:128, 0:B * XC] = xs[2, 0:42].reshape(42, B * XC)
        xlast[0, 0:B * XC] = xs[2, 42].reshape(B * XC)
        xall[0:43, B * XC:] = w1[0].reshape(XR, 600)
        xall[43:86, B * XC:] = w1[1].reshape(XR, 600)
        xall[86:128, B * XC:] = w1[2, 0:42].reshape(42, 600)
        xlast[0, B * XC:] = w1[2, 42].reshape(600)

        # conv2 banded weights: K=(py_loc, ich), M=(i2_loc, och2); bias folded
        # into the eviction activation
        w2 = np.zeros((114, 3, 135), f32)
        for i2_loc in range(C2R):
            i2 = a + i2_loc
            if i2 > 61:
                continue
            for kyp in range(3):
                py_loc = 2 * i2_loc + kyp
                py = Y0 + py_loc
                if py_loc >= P1R or not (0 <= py <= 122):
                    continue
                for ich in range(6):
                    q = py_loc * 6 + ich
                    m0 = i2_loc * 15
                    w2[q, :, m0:m0 + 15] = conv2_w[:, ich, kyp, :].T

        # partition-shift matrices
        s1m = np.zeros((120, 114), f32)
        for m in range(114):
            s1m[m + 6, m] = 1.0
        s2a = np.zeros((120, 120), f32)
        s2b = np.zeros((15, 120), f32)
        for m in range(105):
            s2a[m + 15, m] = 1.0
        for m in range(105, 120):
            s2b[m - 105, m] = 1.0

        # fc1 weight slab: [p=(i2_loc,och2), j, och1]
        wsl = np.zeros((8, 15, NJ, 120), f32)
        nrow = min(nb, 8)
        wsl[0:nrow] = fc1_w4[:, :, a:a + nrow, :].transpose(2, 1, 3, 0)
        wslab = wsl.reshape(120, NJ, 120)

        big2 = np.zeros((120, W2W), f32)
        big2[0:120, 0:114] = s1m
        big2[0:120, 114:234] = s2a
        big2[0:15, 234:354] = s2b
        big2[0:114, 354:759] = w2.reshape(114, 405)
        big2[:, 768:W2W] = wslab.reshape(120, NJ * 120)

        m = dict(shared)
        m.update({"xall": xall.astype(bf),
                  "xlast": xlast.astype(bf),
                  "big2": big2.astype(bf)})
        in_maps.append(m)
    return in_maps


_NC_CACHE = None


def kernel(**inputs) -> np.ndarray:
    global _NC_CACHE
    if _NC_CACHE is None:
        _NC_CACHE = _build_nc()
    nc = _NC_CACHE
    in_maps = _prep_inputs(inputs)
    res = bass_utils.run_bass_kernel_spmd(
        nc, in_maps, core_ids=list(range(N_CORES)))
    return res.results[0]["out"]
